# revision 27
# baseline (speedup 1.0000x reference)
"""AgreementRouting (CapsNet dynamic routing) Trainium2 Bass kernel, v2.

Full input [256, 1152, 10, 16] f32 -> v [256, 10, 16] f32.
Data-parallel over batch: 32 samples per core on 8 cores.

Per-core plan (memory-regime):
  - Load: per-sample HWDGE DMA of contiguous f32 (i permuted so each
    partition reads ONE contiguous 5760B run: i = 9*p + j), staged in SBUF
    f32, converted to resident bf16 Xb by DVE 2x copies.  This replaces the
    old 2-byte-strided bf16 extraction that generated 5.9M descriptors.
  - Phase -1 (s0): einsum2 with constant c=0.1 stationary, col-tiled
    4 samples/PSUM tile, batched squash, matmul collapse+broadcast of v.
  - 3 routing iterations fully on-chip, batches of 4 samples:
      einsum1: bf16 mul (DVE 2x / GpSimd) + bf16 pair-tree reduction over d
               (TT adds at 2x beat tensor_reduce's 1x mode); 2 of 8 batches
               run on GpSimd to unload DVE (the bottleneck engine)
      softmax over o: ACT Exp + DVE sum/recip/mul
      einsum2: PE matmuls, c stationary, 4 samples col-tiled per PSUM tile
      squash: batched over 4 samples on [128,*] tiles; sqrt via exp(.5*ln)
              so ACT stays on the natural_log_exp table set (no reloads)
      v feedback: one [128,128]-ones matmul per sample does collapse AND
              broadcast-to-all-partitions in one shot.
  - All PE dependencies are DVE-produced (PE instrs carry one sem wait).
"""

import numpy as np

import concourse.bacc as bacc
import concourse.bass as bass
import concourse.mybir as mybir
import concourse.tile as tile
from concourse.bass import AP

NCORES = 8
B = 256
S = B // NCORES          # 32 samples per core
I = 1152
O = 10
D = 16
OD = O * D               # 160
P = 128
NJ = I // P              # 9: i = 9*p + j
SW = NJ * OD             # 1440 elems per sample per partition
TB = 4                   # batch: einsum1 group == einsum2 col-tiled samples
NB = S // TB             # 8 batches
GPS_BATCHES = ()         # GpSimd shares DVE's SBUF port: co-running 2-port
                         # DVE ops with GpSimd slows BOTH ~3.5x (measured),
                         # so einsum1 stays entirely on DVE
N_ITER = 3

F32 = mybir.dt.float32
BF16 = mybir.dt.bfloat16
AX = mybir.AxisListType.X
AF = mybir.ActivationFunctionType
ALU = mybir.AluOpType


def _rep_mid(a, n, at=1):
    """Insert a step-0 (broadcast) dim of size n into free dims at position `at`."""
    ap = [list(e) for e in a.ap]
    ap = ap[:at] + [[0, n]] + ap[at:]
    return AP(a.tensor, a.offset, ap)


def _rep_last(a, n):
    ap = [list(e) for e in a.ap] + [[0, n]]
    return AP(a.tensor, a.offset, ap)


def _build():
    nc = bacc.Bacc(None, target_bir_lowering=False)
    x = nc.dram_tensor("x", [S, I, O, D], F32, kind="ExternalInput")
    vout = nc.dram_tensor("v", [S, O, D], F32, kind="ExternalOutput")

    with tile.TileContext(nc) as tc:
        with (
            tc.tile_pool(name="big", bufs=1) as big,
            tc.tile_pool(name="stage", bufs=2) as stagep,
            tc.tile_pool(name="e1", bufs=1) as e1p,      # einsum1 scratch
            tc.tile_pool(name="bi", bufs=2) as bip,
            tc.tile_pool(name="work", bufs=2) as workp,
            tc.tile_pool(name="small", bufs=3) as smallp,
            tc.tile_pool(name="const", bufs=1) as constp,
            tc.tile_pool(name="ps_s", bufs=3, space="PSUM") as ps_sp,
            tc.tile_pool(name="ps_bc", bufs=2, space="PSUM") as ps_bcp,
            tc.tile_pool(name="ps_v", bufs=2, space="PSUM") as ps_vp,
            tc.tile_pool(name="ps_w", bufs=1, space="PSUM") as ps_wp,
        ):
            # ---- persistent state ----
            Xb = big.tile([P, S * SW], BF16)          # bf16 input, (s, j, o, d)
            vb_all = big.tile([P, S * OD], BF16)      # v bcast to all partitions
            b_cum = big.tile([P, S * NJ * O], F32)    # routing logits (s, j, o)
            c_t = big.tile([P, S * NJ * O], BF16)     # softmax coeffs (s, j, o)

            # ---- constants ----
            mask128 = constp.tile([P, OD], F32)       # 4 strips of block-diag o/od
            ones16 = constp.tile([1, D], F32)
            onesrow = constp.tile([32, P], BF16)      # src for A strips
            A = constp.tile([P, TB * P], BF16)        # A[:, t*P:(t+1)*P]: ones on strip t
            c01 = constp.tile([P, O], BF16)           # uniform routing c = 0.1

            nc.vector.memset(ones16[:], 1.0)
            nc.vector.memset(mask128[:], 0.0)
            for t in range(TB):
                for o in range(O):
                    # engines can't write at partition offsets; DMA can
                    nc.sync.dma_start(
                        mask128[32 * t + o : 32 * t + o + 1, o * D : (o + 1) * D],
                        ones16[:],
                    )
            nc.vector.memset(onesrow[:], 1.0)
            nc.vector.memset(A[:], 0.0)
            for t in range(TB):
                nc.sync.dma_start(
                    A[32 * t : 32 * (t + 1), t * P : (t + 1) * P], onesrow[:]
                )
            nc.vector.memset(c01[:], 0.1)

            # PE warmup so PE observes the DVE-const tick once
            warm = ps_wp.tile([1, O], F32, tag="warm")
            nc.tensor.matmul(warm[:], A[:, :1], c01[:], start=True, stop=True)

            def load_batch(q):
                b0 = TB * q
                st = stagep.tile([P, TB * SW], F32)
                nc.sync.dma_start(
                    st[:].rearrange("p (s f) -> p s f", s=TB),
                    x[b0 : b0 + TB].rearrange("s (p j) o d -> p s (j o d)", p=P),
                )
                nc.vector.tensor_copy(Xb[:, b0 * SW : (b0 + TB) * SW], st[:])

            def einsum1_batch(q, k):
                """b_cum[s,i,o] += sum_d X * v for samples 4q..4q+3."""
                on_gps = q in GPS_BATCHES
                eng = nc.gpsimd if on_gps else nc.vector
                sfx = "g" if on_gps else "d"
                s0 = TB * q
                M = TB * NJ * O          # 360 d-groups
                tmp = e1p.tile([P, TB * SW], BF16, tag="tmp" + sfx)
                xs = Xb[:, s0 * SW : (s0 + TB) * SW].rearrange(
                    "p (s j f) -> p s j f", s=TB, j=NJ
                )
                vb = vb_all[:, s0 * OD : (s0 + TB) * OD].rearrange(
                    "p (s f) -> p s f", s=TB
                )
                eng.tensor_mul(
                    tmp[:].rearrange("p (s j f) -> p s j f", s=TB, j=NJ),
                    xs,
                    _rep_mid(vb, NJ, at=2),
                )
                tv = tmp[:].rearrange("p (m e) -> p m e", e=D)
                t1 = e1p.tile([P, M * 8], BF16, tag="t1" + sfx)
                t1v = t1[:].rearrange("p (m e) -> p m e", e=8)
                eng.tensor_add(t1v, tv[:, :, 0:8], tv[:, :, 8:16])
                t2 = e1p.tile([P, M * 4], BF16, tag="t2" + sfx)
                t2v = t2[:].rearrange("p (m e) -> p m e", e=4)
                eng.tensor_add(t2v, t1v[:, :, 0:4], t1v[:, :, 4:8])
                t3 = e1p.tile([P, M * 2], BF16, tag="t3" + sfx)
                t3v = t3[:].rearrange("p (m e) -> p m e", e=2)
                eng.tensor_add(t3v, t2v[:, :, 0:2], t2v[:, :, 2:4])
                bsl = b_cum[:, s0 * NJ * O : (s0 + TB) * NJ * O]
                if k == 0:
                    eng.tensor_add(
                        bsl.rearrange("p (m e) -> p m e", e=1),
                        t3v[:, :, 0:1],
                        t3v[:, :, 1:2],
                    )
                else:
                    binc = bip.tile([P, M], F32, tag="bi" + sfx)
                    eng.tensor_add(
                        binc[:].rearrange("p (m e) -> p m e", e=1),
                        t3v[:, :, 0:1],
                        t3v[:, :, 1:2],
                    )
                    nc.vector.tensor_add(bsl, bsl, binc[:])

            def softmax_batch(q):
                s0 = TB * q
                M = TB * NJ              # 36 softmax rows per partition
                bsl = b_cum[:, s0 * NJ * O : (s0 + TB) * NJ * O]
                e_g = workp.tile([P, TB * NJ * O], F32, tag="eg")
                nc.scalar.activation(e_g[:], bsl, AF.Exp)
                rs = smallp.tile([P, M], F32, tag="rs")
                nc.vector.reduce_sum(
                    rs[:], e_g[:].rearrange("p (m o) -> p m o", o=O), axis=AX
                )
                rr = smallp.tile([P, M], F32, tag="rr")
                nc.vector.reciprocal(rr[:], rs[:])
                nc.vector.tensor_mul(
                    c_t[:, s0 * NJ * O : (s0 + TB) * NJ * O].rearrange(
                        "p (m o) -> p m o", o=O
                    ),
                    e_g[:].rearrange("p (m o) -> p m o", o=O),
                    _rep_last(rr[:], O),
                )

            def einsum2_batch(q, stationary):
                """4 col-tiled samples' s = sum_i c[i,o] X[i,od] -> ps [128,160]."""
                ps = ps_sp.tile([P, OD], F32)
                for j in range(NJ):
                    for t in range(TB):
                        b = TB * q + t
                        if stationary is None:
                            lhsT = c_t[:, (b * NJ + j) * O : (b * NJ + j + 1) * O]
                        else:
                            lhsT = stationary[:]
                        nc.tensor.matmul(
                            ps[32 * t : 32 * t + O, :],
                            lhsT,
                            Xb[:, (b * SW + j * OD) : (b * SW + (j + 1) * OD)],
                            start=(j == 0),
                            stop=(j == NJ - 1),
                            tile_position=(0, 32 * t),
                        )
                return ps

            def squash_start(q, ps):
                """masked s + ACT square/accum -> (masked, l2)."""
                masked = smallp.tile([P, OD], F32, tag="mk")
                nc.vector.tensor_mul(masked[:], ps[:], mask128[:])
                sq = smallp.tile([P, OD], BF16, tag="sq")
                l2 = smallp.tile([P, 1], F32, tag="l2")
                nc.scalar.activation(sq[:], masked[:], AF.Square, accum_out=l2[:])
                return masked, l2

            def squash_finish(q, masked, l2):
                """scale = sqrt(l2)/(1+l2) via DVE bit-trick sqrt + one Newton
                step (keeps ACT on the exp table set; no table reloads)."""
                half_i = smallp.tile([P, 1], F32, tag="hi")
                nc.vector.tensor_scalar(
                    half_i[:].bitcast(mybir.dt.int32),
                    l2[:].bitcast(mybir.dt.int32),
                    1,
                    None,
                    op0=ALU.logical_shift_right,
                )
                rt0 = smallp.tile([P, 1], F32, tag="rt")
                nc.vector.tensor_scalar(
                    rt0[:].bitcast(mybir.dt.int32),
                    half_i[:].bitcast(mybir.dt.int32),
                    0x1FBD1DF5,
                    None,
                    op0=ALU.add,
                )
                q0 = smallp.tile([P, 1], F32, tag="q0")
                nc.vector.reciprocal(q0[:], rt0[:])
                t = smallp.tile([P, 1], F32, tag="t")
                nc.vector.tensor_mul(t[:], l2[:], q0[:])
                num = smallp.tile([P, 1], F32, tag="nm")
                nc.vector.tensor_add(num[:], rt0[:], t[:])
                den2 = smallp.tile([P, 1], F32, tag="d2")
                nc.vector.tensor_scalar(
                    den2[:], l2[:], 2.0, 2.0, op0=ALU.mult, op1=ALU.add
                )
                rden2 = smallp.tile([P, 1], F32, tag="rd")
                nc.vector.reciprocal(rden2[:], den2[:])
                sc = smallp.tile([P, 1], F32, tag="sc")
                nc.vector.tensor_mul(sc[:], num[:], rden2[:])
                v_full = smallp.tile([P, OD], BF16, tag="vf")
                nc.vector.tensor_single_scalar(v_full[:], masked[:], sc[:], op=ALU.mult)
                return v_full

            def broadcast_batch(q, v_full):
                """v per sample -> all partitions of vb_all (collapse+bcast matmul)."""
                for half in range(2):
                    bc = ps_bcp.tile([P, 2 * OD], F32, tag="bc")
                    for u in range(2):
                        t = 2 * half + u
                        nc.tensor.matmul(
                            bc[:, u * OD : (u + 1) * OD],
                            A[:, t * P : (t + 1) * P],
                            v_full[:],
                            start=True,
                            stop=True,
                        )
                    nc.scalar.copy(
                        vb_all[
                            :,
                            (TB * q + 2 * half) * OD : (TB * q + 2 * half + 2) * OD,
                        ],
                        bc[:],
                    )

            def output_batch(q, v_full):
                for t in range(TB):
                    b = TB * q + t
                    psv = ps_vp.tile([1, OD], F32, tag="pv")
                    nc.tensor.matmul(
                        psv[:], A[:, t * P : t * P + 1], v_full[:],
                        start=True, stop=True,
                    )
                    v_sb = smallp.tile([1, OD], F32, tag="vo")
                    nc.scalar.copy(v_sb[:], psv[:])
                    nc.sync.dma_start(
                        vout[b : b + 1].rearrange("b o d -> b (o d)"), v_sb[:]
                    )

            # Deferred squash finishes: software pipeline so the DVE fills the
            # ACT-square latency of batch q with batch q+1's einsum1 work.
            pending = []

            def flush_pending():
                items = pending[:]
                pending.clear()
                for kind, fq, fm, fl2, flast in items:
                    vf = squash_finish(fq, fm, fl2)
                    if flast:
                        output_batch(fq, vf)
                    else:
                        broadcast_batch(fq, vf)
                    if kind == "s0":
                        # launch iteration 0 for this batch, finish deferred
                        einsum1_batch(fq, 0)
                        softmax_batch(fq)
                        ps = einsum2_batch(fq, None)
                        m, l2 = squash_start(fq, ps)
                        pending.append(("it", fq, m, l2, False))

            # ---- phase -1 (load + s0) merged with iteration 0 ----
            for q in range(NB):
                load_batch(q)
                ps = einsum2_batch(q, c01)
                m, l2 = squash_start(q, ps)
                flush_pending()
                pending.append(("s0", q, m, l2, False))
            flush_pending()
            flush_pending()

            # ---- iterations 1..2 ----
            for k in range(1, N_ITER):
                last = k == N_ITER - 1
                for q in range(NB):
                    einsum1_batch(q, k)
                    softmax_batch(q)
                    ps = einsum2_batch(q, None)
                    flush_pending()
                    m, l2 = squash_start(q, ps)
                    pending.append(("it", q, m, l2, last))
                flush_pending()

    nc.compile()
    return nc


_cached = {}


def _get_nc():
    if "nc" not in _cached:
        _cached["nc"] = _build()
    return _cached["nc"]


def kernel(input, _trace=False):
    from concourse.bass_utils import run_bass_kernel_spmd

    input = np.ascontiguousarray(np.asarray(input, dtype=np.float32))
    assert input.shape == (B, I, O, D)
    nc = _get_nc()
    in_maps = [{"x": input[c * S : (c + 1) * S]} for c in range(NCORES)]
    res = run_bass_kernel_spmd(
        nc, in_maps, core_ids=list(range(NCORES)), trace=_trace
    )
    out = np.concatenate([r["v"] for r in res.results], axis=0)
    if _trace:
        kernel.last_exec_time_ns = res.exec_time_ns
        kernel.last_res = res
    return out.astype(np.float32)


kernel.last_exec_time_ns = None


# revision 28
# speedup vs baseline: 1.0261x; 1.0261x over previous
"""AgreementRouting (CapsNet dynamic routing) Trainium2 Bass kernel, v2.

Full input [256, 1152, 10, 16] f32 -> v [256, 10, 16] f32.
Data-parallel over batch: 32 samples per core on 8 cores.

Per-core plan (memory-regime):
  - Load: per-sample HWDGE DMA of contiguous f32 (i permuted so each
    partition reads ONE contiguous 5760B run: i = 9*p + j), staged in SBUF
    f32, converted to resident bf16 Xb by DVE 2x copies.  This replaces the
    old 2-byte-strided bf16 extraction that generated 5.9M descriptors.
  - Phase -1 (s0): einsum2 with constant c=0.1 stationary, col-tiled
    4 samples/PSUM tile, batched squash, matmul collapse+broadcast of v.
  - 3 routing iterations fully on-chip, batches of 4 samples:
      einsum1: bf16 mul (DVE 2x / GpSimd) + bf16 pair-tree reduction over d
               (TT adds at 2x beat tensor_reduce's 1x mode); 2 of 8 batches
               run on GpSimd to unload DVE (the bottleneck engine)
      softmax over o: ACT Exp + DVE sum/recip/mul
      einsum2: PE matmuls, c stationary, 4 samples col-tiled per PSUM tile
      squash: batched over 4 samples on [128,*] tiles; sqrt via exp(.5*ln)
              so ACT stays on the natural_log_exp table set (no reloads)
      v feedback: one [128,128]-ones matmul per sample does collapse AND
              broadcast-to-all-partitions in one shot.
  - All PE dependencies are DVE-produced (PE instrs carry one sem wait).
"""

import numpy as np

import concourse.bacc as bacc
import concourse.bass as bass
import concourse.mybir as mybir
import concourse.tile as tile
from concourse.bass import AP

NCORES = 8
B = 256
S = B // NCORES          # 32 samples per core
I = 1152
O = 10
D = 16
OD = O * D               # 160
P = 128
NJ = I // P              # 9: i = 9*p + j
SW = NJ * OD             # 1440 elems per sample per partition
TB = 4                   # batch: einsum1 group == einsum2 col-tiled samples
NB = S // TB             # 8 batches
GPS_BATCHES = ()         # GpSimd shares DVE's SBUF port: co-running 2-port
                         # DVE ops with GpSimd slows BOTH ~3.5x (measured),
                         # so einsum1 stays entirely on DVE
N_ITER = 3

F32 = mybir.dt.float32
BF16 = mybir.dt.bfloat16
AX = mybir.AxisListType.X
AF = mybir.ActivationFunctionType
ALU = mybir.AluOpType


def _rep_mid(a, n, at=1):
    """Insert a step-0 (broadcast) dim of size n into free dims at position `at`."""
    ap = [list(e) for e in a.ap]
    ap = ap[:at] + [[0, n]] + ap[at:]
    return AP(a.tensor, a.offset, ap)


def _rep_last(a, n):
    ap = [list(e) for e in a.ap] + [[0, n]]
    return AP(a.tensor, a.offset, ap)


def _build():
    nc = bacc.Bacc(None, target_bir_lowering=False)
    x = nc.dram_tensor("x", [S, I, O, D], F32, kind="ExternalInput")
    vout = nc.dram_tensor("v", [S, O, D], F32, kind="ExternalOutput")

    with tile.TileContext(nc) as tc:
        with (
            tc.tile_pool(name="big", bufs=1) as big,
            tc.tile_pool(name="stage", bufs=2) as stagep,
            tc.tile_pool(name="e1", bufs=1) as e1p,      # einsum1 scratch
            tc.tile_pool(name="bi", bufs=2) as bip,
            tc.tile_pool(name="work", bufs=2) as workp,
            tc.tile_pool(name="small", bufs=3) as smallp,
            tc.tile_pool(name="const", bufs=1) as constp,
            tc.tile_pool(name="ps_s", bufs=3, space="PSUM") as ps_sp,
            tc.tile_pool(name="ps_bc", bufs=2, space="PSUM") as ps_bcp,
            tc.tile_pool(name="ps_v", bufs=2, space="PSUM") as ps_vp,
            tc.tile_pool(name="ps_w", bufs=1, space="PSUM") as ps_wp,
        ):
            # ---- persistent state ----
            Xb = big.tile([P, S * SW], BF16)          # bf16 input, (s, j, o, d)
            vb_all = big.tile([P, S * OD], BF16)      # v bcast to all partitions
            b_cum = big.tile([P, S * NJ * O], F32)    # routing logits (s, j, o)
            c_t = big.tile([P, S * NJ * O], BF16)     # softmax coeffs (s, j, o)

            # ---- constants ----
            mask128 = constp.tile([P, OD], F32)       # 4 strips of block-diag o/od
            ones16 = constp.tile([1, D], F32)
            onesrow = constp.tile([32, P], BF16)      # src for A strips
            A = constp.tile([P, TB * P], BF16)        # A[:, t*P:(t+1)*P]: ones on strip t
            c01 = constp.tile([P, O], BF16)           # uniform routing c = 0.1

            nc.vector.memset(ones16[:], 1.0)
            nc.vector.memset(mask128[:], 0.0)
            for t in range(TB):
                for o in range(O):
                    # engines can't write at partition offsets; DMA can
                    nc.sync.dma_start(
                        mask128[32 * t + o : 32 * t + o + 1, o * D : (o + 1) * D],
                        ones16[:],
                    )
            nc.vector.memset(onesrow[:], 1.0)
            nc.vector.memset(A[:], 0.0)
            for t in range(TB):
                nc.sync.dma_start(
                    A[32 * t : 32 * (t + 1), t * P : (t + 1) * P], onesrow[:]
                )
            nc.vector.memset(c01[:], 0.1)

            # PE warmup so PE observes the DVE-const tick once
            warm = ps_wp.tile([1, O], F32, tag="warm")
            nc.tensor.matmul(warm[:], A[:, :1], c01[:], start=True, stop=True)

            def load_batch(q):
                b0 = TB * q
                st = stagep.tile([P, TB * SW], F32)
                nc.sync.dma_start(
                    st[:].rearrange("p (s f) -> p s f", s=TB),
                    x[b0 : b0 + TB].rearrange("s (p j) o d -> p s (j o d)", p=P),
                )
                nc.vector.tensor_copy(Xb[:, b0 * SW : (b0 + TB) * SW], st[:])

            def einsum1_batch(q, k):
                """b_cum[s,i,o] += sum_d X * v for samples 4q..4q+3."""
                on_gps = q in GPS_BATCHES
                eng = nc.gpsimd if on_gps else nc.vector
                sfx = "g" if on_gps else "d"
                s0 = TB * q
                M = TB * NJ * O          # 360 d-groups
                tmp = e1p.tile([P, TB * SW], BF16, tag="tmp" + sfx)
                xs = Xb[:, s0 * SW : (s0 + TB) * SW].rearrange(
                    "p (s j f) -> p s j f", s=TB, j=NJ
                )
                vb = vb_all[:, s0 * OD : (s0 + TB) * OD].rearrange(
                    "p (s f) -> p s f", s=TB
                )
                eng.tensor_mul(
                    tmp[:].rearrange("p (s j f) -> p s j f", s=TB, j=NJ),
                    xs,
                    _rep_mid(vb, NJ, at=2),
                )
                tv = tmp[:].rearrange("p (m e) -> p m e", e=D)
                t1 = e1p.tile([P, M * 8], BF16, tag="t1" + sfx)
                t1v = t1[:].rearrange("p (m e) -> p m e", e=8)
                eng.tensor_add(t1v, tv[:, :, 0:8], tv[:, :, 8:16])
                t2 = e1p.tile([P, M * 4], BF16, tag="t2" + sfx)
                t2v = t2[:].rearrange("p (m e) -> p m e", e=4)
                eng.tensor_add(t2v, t1v[:, :, 0:4], t1v[:, :, 4:8])
                t3 = e1p.tile([P, M * 2], BF16, tag="t3" + sfx)
                t3v = t3[:].rearrange("p (m e) -> p m e", e=2)
                eng.tensor_add(t3v, t2v[:, :, 0:2], t2v[:, :, 2:4])
                bsl = b_cum[:, s0 * NJ * O : (s0 + TB) * NJ * O]
                if k == 0:
                    eng.tensor_add(
                        bsl.rearrange("p (m e) -> p m e", e=1),
                        t3v[:, :, 0:1],
                        t3v[:, :, 1:2],
                    )
                else:
                    binc = bip.tile([P, M], F32, tag="bi" + sfx)
                    eng.tensor_add(
                        binc[:].rearrange("p (m e) -> p m e", e=1),
                        t3v[:, :, 0:1],
                        t3v[:, :, 1:2],
                    )
                    nc.vector.tensor_add(bsl, bsl, binc[:])

            def softmax_batch(q):
                s0 = TB * q
                M = TB * NJ              # 36 softmax rows per partition
                bsl = b_cum[:, s0 * NJ * O : (s0 + TB) * NJ * O]
                e_g = workp.tile([P, TB * NJ * O], F32, tag="eg")
                nc.scalar.activation(e_g[:], bsl, AF.Exp)
                rs = smallp.tile([P, M], F32, tag="rs")
                nc.vector.reduce_sum(
                    rs[:], e_g[:].rearrange("p (m o) -> p m o", o=O), axis=AX
                )
                rr = smallp.tile([P, M], F32, tag="rr")
                nc.vector.reciprocal(rr[:], rs[:])
                nc.vector.tensor_mul(
                    c_t[:, s0 * NJ * O : (s0 + TB) * NJ * O].rearrange(
                        "p (m o) -> p m o", o=O
                    ),
                    e_g[:].rearrange("p (m o) -> p m o", o=O),
                    _rep_last(rr[:], O),
                )

            def einsum2_batch(q, stationary):
                """4 col-tiled samples' s = sum_i c[i,o] X[i,od] -> ps [128,160]."""
                ps = ps_sp.tile([P, OD], F32)
                for j in range(NJ):
                    for t in range(TB):
                        b = TB * q + t
                        if stationary is None:
                            lhsT = c_t[:, (b * NJ + j) * O : (b * NJ + j + 1) * O]
                        else:
                            lhsT = stationary[:]
                        nc.tensor.matmul(
                            ps[32 * t : 32 * t + O, :],
                            lhsT,
                            Xb[:, (b * SW + j * OD) : (b * SW + (j + 1) * OD)],
                            start=(j == 0),
                            stop=(j == NJ - 1),
                            tile_position=(0, 32 * t),
                        )
                return ps

            def squash_start(q, ps):
                """masked s + ACT square/accum -> (masked, l2)."""
                masked = smallp.tile([P, OD], F32, tag="mk")
                nc.vector.tensor_mul(masked[:], ps[:], mask128[:])
                sq = smallp.tile([P, OD], BF16, tag="sq")
                l2 = smallp.tile([P, 1], F32, tag="l2")
                nc.scalar.activation(sq[:], masked[:], AF.Square, accum_out=l2[:])
                return masked, l2

            def squash_finish(q, masked, l2):
                """scale = sqrt(l2)/(1+l2) via DVE bit-trick sqrt + one Newton
                step (keeps ACT on the exp table set; no table reloads)."""
                half_i = smallp.tile([P, 1], F32, tag="hi")
                nc.vector.tensor_scalar(
                    half_i[:].bitcast(mybir.dt.int32),
                    l2[:].bitcast(mybir.dt.int32),
                    1,
                    None,
                    op0=ALU.logical_shift_right,
                )
                rt0 = smallp.tile([P, 1], F32, tag="rt")
                nc.vector.tensor_scalar(
                    rt0[:].bitcast(mybir.dt.int32),
                    half_i[:].bitcast(mybir.dt.int32),
                    0x1FBD1DF5,
                    None,
                    op0=ALU.add,
                )
                q0 = smallp.tile([P, 1], F32, tag="q0")
                nc.vector.reciprocal(q0[:], rt0[:])
                t = smallp.tile([P, 1], F32, tag="t")
                nc.vector.tensor_mul(t[:], l2[:], q0[:])
                num = smallp.tile([P, 1], F32, tag="nm")
                nc.vector.tensor_add(num[:], rt0[:], t[:])
                den2 = smallp.tile([P, 1], F32, tag="d2")
                nc.vector.tensor_scalar(
                    den2[:], l2[:], 2.0, 2.0, op0=ALU.mult, op1=ALU.add
                )
                rden2 = smallp.tile([P, 1], F32, tag="rd")
                nc.vector.reciprocal(rden2[:], den2[:])
                sc = smallp.tile([P, 1], F32, tag="sc")
                nc.vector.tensor_mul(sc[:], num[:], rden2[:])
                v_full = smallp.tile([P, OD], BF16, tag="vf")
                nc.vector.tensor_single_scalar(v_full[:], masked[:], sc[:], op=ALU.mult)
                return v_full

            def broadcast_batch(q, v_full):
                """v per sample -> all partitions of vb_all (collapse+bcast matmul)."""
                for half in range(2):
                    bc = ps_bcp.tile([P, 2 * OD], F32, tag="bc")
                    for u in range(2):
                        t = 2 * half + u
                        nc.tensor.matmul(
                            bc[:, u * OD : (u + 1) * OD],
                            A[:, t * P : (t + 1) * P],
                            v_full[:],
                            start=True,
                            stop=True,
                        )
                    nc.scalar.copy(
                        vb_all[
                            :,
                            (TB * q + 2 * half) * OD : (TB * q + 2 * half + 2) * OD,
                        ],
                        bc[:],
                    )

            def output_batch(q, v_full):
                for t in range(TB):
                    b = TB * q + t
                    psv = ps_vp.tile([1, OD], F32, tag="pv")
                    nc.tensor.matmul(
                        psv[:], A[:, t * P : t * P + 1], v_full[:],
                        start=True, stop=True,
                    )
                    v_sb = smallp.tile([1, OD], F32, tag="vo")
                    nc.scalar.copy(v_sb[:], psv[:])
                    nc.sync.dma_start(
                        vout[b : b + 1].rearrange("b o d -> b (o d)"), v_sb[:]
                    )

            # Deferred squash finishes: software pipeline so the DVE fills the
            # ACT-square latency of batch q with batch q+1's einsum1 work.
            pending = []

            def flush_pending():
                items = pending[:]
                pending.clear()
                for kind, fq, fm, fl2, flast in items:
                    vf = squash_finish(fq, fm, fl2)
                    if flast:
                        output_batch(fq, vf)
                    else:
                        broadcast_batch(fq, vf)

            # ---- phase -1 (load + s0) merged with iteration 0 ----
            for q in range(NB):
                load_batch(q)
                ps = einsum2_batch(q, c01)
                m, l2 = squash_start(q, ps)
                vf = squash_finish(q, m, l2)
                broadcast_batch(q, vf)
                flush_pending()
                einsum1_batch(q, 0)
                softmax_batch(q)
                ps = einsum2_batch(q, None)
                m, l2 = squash_start(q, ps)
                pending.append(("it", q, m, l2, False))
            flush_pending()

            # ---- iterations 1..2 ----
            for k in range(1, N_ITER):
                last = k == N_ITER - 1
                for q in range(NB):
                    einsum1_batch(q, k)
                    softmax_batch(q)
                    ps = einsum2_batch(q, None)
                    flush_pending()
                    m, l2 = squash_start(q, ps)
                    pending.append(("it", q, m, l2, last))
                flush_pending()

    nc.compile()
    return nc


_cached = {}


def _get_nc():
    if "nc" not in _cached:
        _cached["nc"] = _build()
    return _cached["nc"]


def kernel(input, _trace=False):
    from concourse.bass_utils import run_bass_kernel_spmd

    input = np.ascontiguousarray(np.asarray(input, dtype=np.float32))
    assert input.shape == (B, I, O, D)
    nc = _get_nc()
    in_maps = [{"x": input[c * S : (c + 1) * S]} for c in range(NCORES)]
    res = run_bass_kernel_spmd(
        nc, in_maps, core_ids=list(range(NCORES)), trace=_trace
    )
    out = np.concatenate([r["v"] for r in res.results], axis=0)
    if _trace:
        kernel.last_exec_time_ns = res.exec_time_ns
        kernel.last_res = res
    return out.astype(np.float32)


kernel.last_exec_time_ns = None


# revision 30
# speedup vs baseline: 1.2168x; 1.1858x over previous
"""AgreementRouting (CapsNet dynamic routing) Trainium2 Bass kernel, v2.

Full input [256, 1152, 10, 16] f32 -> v [256, 10, 16] f32.
Data-parallel over batch: 32 samples per core on 8 cores.

Per-core plan (memory-regime):
  - Load: per-sample HWDGE DMA of contiguous f32 (i permuted so each
    partition reads ONE contiguous 5760B run: i = 9*p + j), staged in SBUF
    f32, converted to resident bf16 Xb by DVE 2x copies.  This replaces the
    old 2-byte-strided bf16 extraction that generated 5.9M descriptors.
  - Phase -1 (s0): einsum2 with constant c=0.1 stationary, col-tiled
    4 samples/PSUM tile, batched squash, matmul collapse+broadcast of v.
  - 3 routing iterations fully on-chip, batches of 4 samples:
      einsum1: bf16 mul (DVE 2x / GpSimd) + bf16 pair-tree reduction over d
               (TT adds at 2x beat tensor_reduce's 1x mode); 2 of 8 batches
               run on GpSimd to unload DVE (the bottleneck engine)
      softmax over o: ACT Exp + DVE sum/recip/mul
      einsum2: PE matmuls, c stationary, 4 samples col-tiled per PSUM tile
      squash: batched over 4 samples on [128,*] tiles; sqrt via exp(.5*ln)
              so ACT stays on the natural_log_exp table set (no reloads)
      v feedback: one [128,128]-ones matmul per sample does collapse AND
              broadcast-to-all-partitions in one shot.
  - All PE dependencies are DVE-produced (PE instrs carry one sem wait).
"""

import numpy as np

import concourse.bacc as bacc
import concourse.bass as bass
import concourse.mybir as mybir
import concourse.tile as tile
from concourse.bass import AP

NCORES = 8
B = 256
S = B // NCORES          # 32 samples per core
I = 1152
O = 10
D = 16
OD = O * D               # 160
P = 128
NJ = I // P              # 9: i = 9*p + j
SW = NJ * OD             # 1440 elems per sample per partition
TB = 4                   # batch: einsum1 group == einsum2 col-tiled samples
NB = S // TB             # 8 batches
GPS_BATCHES = ()         # GpSimd shares DVE's SBUF port: co-running 2-port
                         # DVE ops with GpSimd slows BOTH ~3.5x (measured),
                         # so einsum1 stays entirely on DVE
N_ITER = 3

F32 = mybir.dt.float32
BF16 = mybir.dt.bfloat16
AX = mybir.AxisListType.X
AF = mybir.ActivationFunctionType
ALU = mybir.AluOpType


def _rep_mid(a, n, at=1):
    """Insert a step-0 (broadcast) dim of size n into free dims at position `at`."""
    ap = [list(e) for e in a.ap]
    ap = ap[:at] + [[0, n]] + ap[at:]
    return AP(a.tensor, a.offset, ap)


def _rep_last(a, n):
    ap = [list(e) for e in a.ap] + [[0, n]]
    return AP(a.tensor, a.offset, ap)


def _build():
    nc = bacc.Bacc(None, target_bir_lowering=False)
    x = nc.dram_tensor("x", [S, I, O, D], F32, kind="ExternalInput")
    vout = nc.dram_tensor("v", [S, O, D], F32, kind="ExternalOutput")

    with tile.TileContext(nc) as tc:
        with (
            tc.tile_pool(name="big", bufs=1) as big,
            tc.tile_pool(name="stage", bufs=2) as stagep,
            tc.tile_pool(name="e1", bufs=1) as e1p,      # einsum1 scratch
            tc.tile_pool(name="bi", bufs=2) as bip,
            tc.tile_pool(name="work", bufs=2) as workp,
            tc.tile_pool(name="small", bufs=3) as smallp,
            tc.tile_pool(name="const", bufs=1) as constp,
            tc.tile_pool(name="ps_s", bufs=3, space="PSUM") as ps_sp,
            tc.tile_pool(name="ps_bc", bufs=2, space="PSUM") as ps_bcp,
            tc.tile_pool(name="ps_v", bufs=2, space="PSUM") as ps_vp,
            tc.tile_pool(name="ps_w", bufs=1, space="PSUM") as ps_wp,
        ):
            # ---- persistent state ----
            Xb = big.tile([P, S * SW], BF16)          # bf16 input, (s, j, o, d)
            vb_all = big.tile([P, S * OD], BF16)      # v bcast to all partitions
            b_cum = big.tile([P, S * NJ * O], F32)    # routing logits (s, j, o)
            c_t = big.tile([P, S * NJ * O], BF16)     # softmax coeffs (s, j, o)

            # ---- constants ----
            mask128 = constp.tile([P, OD], F32)       # 4 strips of block-diag o/od
            ones16 = constp.tile([1, D], F32)
            onesrow = constp.tile([32, P], BF16)      # src for A strips
            A = constp.tile([P, TB * P], BF16)        # A[:, t*P:(t+1)*P]: ones on strip t
            c01 = constp.tile([P, O], BF16)           # uniform routing c = 0.1

            nc.vector.memset(ones16[:], 1.0)
            nc.vector.memset(mask128[:], 0.0)
            for t in range(TB):
                for o in range(O):
                    # engines can't write at partition offsets; DMA can
                    nc.gpsimd.dma_start(
                        mask128[32 * t + o : 32 * t + o + 1, o * D : (o + 1) * D],
                        ones16[:],
                    )
            nc.vector.memset(onesrow[:], 1.0)
            nc.vector.memset(A[:], 0.0)
            for t in range(TB):
                nc.gpsimd.dma_start(
                    A[32 * t : 32 * (t + 1), t * P : (t + 1) * P], onesrow[:]
                )
            nc.vector.memset(c01[:], 0.1)

            # PE warmup so PE observes the DVE-const tick once
            warm = ps_wp.tile([1, O], F32, tag="warm")
            nc.tensor.matmul(warm[:], A[:, :1], c01[:], start=True, stop=True)

            def load_batch(q):
                b0 = TB * q
                st = stagep.tile([P, TB * SW], F32)
                nc.sync.dma_start(
                    st[:].rearrange("p (s f) -> p s f", s=TB),
                    x[b0 : b0 + TB].rearrange("s (p j) o d -> p s (j o d)", p=P),
                )
                nc.vector.tensor_copy(Xb[:, b0 * SW : (b0 + TB) * SW], st[:])

            def einsum1_batch(q, k):
                """b_cum[s,i,o] += sum_d X * v for samples 4q..4q+3."""
                on_gps = q in GPS_BATCHES
                eng = nc.gpsimd if on_gps else nc.vector
                sfx = "g" if on_gps else "d"
                s0 = TB * q
                M = TB * NJ * O          # 360 d-groups
                tmp = e1p.tile([P, TB * SW], BF16, tag="tmp" + sfx)
                xs = Xb[:, s0 * SW : (s0 + TB) * SW].rearrange(
                    "p (s j f) -> p s j f", s=TB, j=NJ
                )
                vb = vb_all[:, s0 * OD : (s0 + TB) * OD].rearrange(
                    "p (s f) -> p s f", s=TB
                )
                eng.tensor_mul(
                    tmp[:].rearrange("p (s j f) -> p s j f", s=TB, j=NJ),
                    xs,
                    _rep_mid(vb, NJ, at=2),
                )
                tv = tmp[:].rearrange("p (m e) -> p m e", e=D)
                t1 = e1p.tile([P, M * 8], BF16, tag="t1" + sfx)
                t1v = t1[:].rearrange("p (m e) -> p m e", e=8)
                eng.tensor_add(t1v, tv[:, :, 0:8], tv[:, :, 8:16])
                t2 = e1p.tile([P, M * 4], BF16, tag="t2" + sfx)
                t2v = t2[:].rearrange("p (m e) -> p m e", e=4)
                eng.tensor_add(t2v, t1v[:, :, 0:4], t1v[:, :, 4:8])
                t3 = e1p.tile([P, M * 2], BF16, tag="t3" + sfx)
                t3v = t3[:].rearrange("p (m e) -> p m e", e=2)
                eng.tensor_add(t3v, t2v[:, :, 0:2], t2v[:, :, 2:4])
                bsl = b_cum[:, s0 * NJ * O : (s0 + TB) * NJ * O]
                if k == 0:
                    eng.tensor_add(
                        bsl.rearrange("p (m e) -> p m e", e=1),
                        t3v[:, :, 0:1],
                        t3v[:, :, 1:2],
                    )
                else:
                    binc = bip.tile([P, M], F32, tag="bi" + sfx)
                    eng.tensor_add(
                        binc[:].rearrange("p (m e) -> p m e", e=1),
                        t3v[:, :, 0:1],
                        t3v[:, :, 1:2],
                    )
                    nc.vector.tensor_add(bsl, bsl, binc[:])

            def softmax_batch(q):
                s0 = TB * q
                M = TB * NJ              # 36 softmax rows per partition
                bsl = b_cum[:, s0 * NJ * O : (s0 + TB) * NJ * O]
                e_g = workp.tile([P, TB * NJ * O], F32, tag="eg")
                nc.scalar.activation(e_g[:], bsl, AF.Exp)
                rs = smallp.tile([P, M], F32, tag="rs")
                nc.vector.reduce_sum(
                    rs[:], e_g[:].rearrange("p (m o) -> p m o", o=O), axis=AX
                )
                rr = smallp.tile([P, M], F32, tag="rr")
                nc.vector.reciprocal(rr[:], rs[:])
                nc.vector.tensor_mul(
                    c_t[:, s0 * NJ * O : (s0 + TB) * NJ * O].rearrange(
                        "p (m o) -> p m o", o=O
                    ),
                    e_g[:].rearrange("p (m o) -> p m o", o=O),
                    _rep_last(rr[:], O),
                )

            def einsum2_batch(q, stationary):
                """4 col-tiled samples' s = sum_i c[i,o] X[i,od] -> ps [128,160]."""
                ps = ps_sp.tile([P, OD], F32)
                for j in range(NJ):
                    for t in range(TB):
                        b = TB * q + t
                        if stationary is None:
                            lhsT = c_t[:, (b * NJ + j) * O : (b * NJ + j + 1) * O]
                        else:
                            lhsT = stationary[:]
                        nc.tensor.matmul(
                            ps[32 * t : 32 * t + O, :],
                            lhsT,
                            Xb[:, (b * SW + j * OD) : (b * SW + (j + 1) * OD)],
                            start=(j == 0),
                            stop=(j == NJ - 1),
                            tile_position=(0, 32 * t),
                        )
                return ps

            def squash_start(q, ps):
                """masked s + ACT square/accum -> (masked, l2)."""
                masked = smallp.tile([P, OD], F32, tag="mk")
                nc.vector.tensor_mul(masked[:], ps[:], mask128[:])
                sq = smallp.tile([P, OD], BF16, tag="sq")
                l2 = smallp.tile([P, 1], F32, tag="l2")
                nc.scalar.activation(sq[:], masked[:], AF.Square, accum_out=l2[:])
                return masked, l2

            def squash_finish(q, masked, l2):
                """scale = sqrt(l2)/(1+l2) via DVE bit-trick sqrt + one Newton
                step (keeps ACT on the exp table set; no table reloads)."""
                half_i = smallp.tile([P, 1], F32, tag="hi")
                nc.vector.tensor_scalar(
                    half_i[:].bitcast(mybir.dt.int32),
                    l2[:].bitcast(mybir.dt.int32),
                    1,
                    None,
                    op0=ALU.logical_shift_right,
                )
                rt0 = smallp.tile([P, 1], F32, tag="rt")
                nc.vector.tensor_scalar(
                    rt0[:].bitcast(mybir.dt.int32),
                    half_i[:].bitcast(mybir.dt.int32),
                    0x1FBD1DF5,
                    None,
                    op0=ALU.add,
                )
                q0 = smallp.tile([P, 1], F32, tag="q0")
                nc.vector.reciprocal(q0[:], rt0[:])
                t = smallp.tile([P, 1], F32, tag="t")
                nc.vector.tensor_mul(t[:], l2[:], q0[:])
                num = smallp.tile([P, 1], F32, tag="nm")
                nc.vector.tensor_add(num[:], rt0[:], t[:])
                den2 = smallp.tile([P, 1], F32, tag="d2")
                nc.vector.tensor_scalar(
                    den2[:], l2[:], 2.0, 2.0, op0=ALU.mult, op1=ALU.add
                )
                rden2 = smallp.tile([P, 1], F32, tag="rd")
                nc.vector.reciprocal(rden2[:], den2[:])
                sc = smallp.tile([P, 1], F32, tag="sc")
                nc.vector.tensor_mul(sc[:], num[:], rden2[:])
                v_full = smallp.tile([P, OD], BF16, tag="vf")
                nc.vector.tensor_single_scalar(v_full[:], masked[:], sc[:], op=ALU.mult)
                return v_full

            def broadcast_batch(q, v_full):
                """v per sample -> all partitions of vb_all (collapse+bcast matmul)."""
                for half in range(2):
                    bc = ps_bcp.tile([P, 2 * OD], F32, tag="bc")
                    for u in range(2):
                        t = 2 * half + u
                        nc.tensor.matmul(
                            bc[:, u * OD : (u + 1) * OD],
                            A[:, t * P : (t + 1) * P],
                            v_full[:],
                            start=True,
                            stop=True,
                        )
                    nc.scalar.copy(
                        vb_all[
                            :,
                            (TB * q + 2 * half) * OD : (TB * q + 2 * half + 2) * OD,
                        ],
                        bc[:],
                    )

            def output_batch(q, v_full):
                b0 = TB * q
                v_sb = smallp.tile([1, TB * OD], F32, tag="vo")
                for t in range(TB):
                    psv = ps_vp.tile([1, OD], F32, tag="pv")
                    nc.tensor.matmul(
                        psv[:], A[:, t * P : t * P + 1], v_full[:],
                        start=True, stop=True,
                    )
                    nc.scalar.copy(v_sb[:, t * OD : (t + 1) * OD], psv[:])
                nc.sync.dma_start(
                    vout[b0 : b0 + TB].rearrange("b o d -> (b o d)").unsqueeze(0),
                    v_sb[:],
                )

            # Deferred squash finishes: software pipeline so the DVE fills the
            # ACT-square latency of batch q with batch q+1's einsum1 work.
            pending = []

            def flush_pending():
                items = pending[:]
                pending.clear()
                for kind, fq, fm, fl2, flast in items:
                    vf = squash_finish(fq, fm, fl2)
                    if flast:
                        output_batch(fq, vf)
                    else:
                        broadcast_batch(fq, vf)

            # ---- phase -1 (load + s0) merged with iteration 0 ----
            for q in range(NB):
                load_batch(q)
                ps = einsum2_batch(q, c01)
                m, l2 = squash_start(q, ps)
                vf = squash_finish(q, m, l2)
                broadcast_batch(q, vf)
                flush_pending()
                einsum1_batch(q, 0)
                softmax_batch(q)
                ps = einsum2_batch(q, None)
                m, l2 = squash_start(q, ps)
                pending.append(("it", q, m, l2, False))
            flush_pending()

            # ---- iterations 1..2 ----
            # 2-deep pipeline: slot q runs finish(q-2) and start(q-1) so the
            # masked-mul never stalls on the PE einsum2 of its own batch.
            sA = []
            for k in range(1, N_ITER):
                last = k == N_ITER - 1
                for q in range(NB):
                    einsum1_batch(q, k)
                    softmax_batch(q)
                    ps = einsum2_batch(q, None)
                    flush_pending()
                    if sA:
                        pq, pps, pl = sA.pop(0)
                        m, l2 = squash_start(pq, pps)
                        pending.append(("it", pq, m, l2, pl))
                    sA.append((q, ps, last))
            while sA or pending:
                flush_pending()
                if sA:
                    pq, pps, pl = sA.pop(0)
                    m, l2 = squash_start(pq, pps)
                    pending.append(("it", pq, m, l2, pl))

    nc.compile()
    return nc


_cached = {}


def _get_nc():
    if "nc" not in _cached:
        _cached["nc"] = _build()
    return _cached["nc"]


def kernel(input, _trace=False):
    from concourse.bass_utils import run_bass_kernel_spmd

    input = np.ascontiguousarray(np.asarray(input, dtype=np.float32))
    assert input.shape == (B, I, O, D)
    nc = _get_nc()
    in_maps = [{"x": input[c * S : (c + 1) * S]} for c in range(NCORES)]
    res = run_bass_kernel_spmd(
        nc, in_maps, core_ids=list(range(NCORES)), trace=_trace
    )
    out = np.concatenate([r["v"] for r in res.results], axis=0)
    if _trace:
        kernel.last_exec_time_ns = res.exec_time_ns
        kernel.last_res = res
    return out.astype(np.float32)


kernel.last_exec_time_ns = None
